# revision 63
# baseline (speedup 1.0000x reference)
"""Multi-head attention (B=2, S=2048, H=1024, NH=16, DK=DV=64) on 8 TRN2 cores.

Sharding: data-parallel over batch (2 groups of 4 cores) x tensor-parallel
over heads (4 heads per core, as 2 pairs of 2).  Each core computes, for its
batch sample and its 4 heads:

    Q^T/K^T projections (features on partitions), V projection (natural),
    S^T = K @ Q^T per 128-key chunk (causal chunks only); the two heads'
    matmuls are independent 64x128 row tiles (T0/T8) that execute
    concurrently on the PE array (~3ns start offset),
    P^T = exp(S^T/8 + pad_bias)   (one fused exp over both heads; no
    max-subtraction needed: |scores| ~ N(0,1)),
    out^T = V_aug^T @ P^T  where V_aug = [V | ones] for head A and
    [ones | V] for head B, so oA = [attnA; denA] and oB = [denB; attnB].
    A single PE matmul with a half-swap permutation aligns both
    denominators with their numerators; one reciprocal + two muls
    normalize directly into attnT (bf16).
    y_partial = attnT^T @ W_O_rows   (row-sharded W_O, all bf16).

Host sums the 4 partials per batch and adds (b_V @ W_O + b_O) (exact since
softmax rows sum to 1).

Pipelining: x^T is uploaded host-pre-arranged as [128, NJ, NCH, 512] so each
query block is one contiguous 1MB DMA region; a handful of dummy warm-up
matmuls at program start bring the PE HAM clock to 2.4 GHz before real work;
projections for query block jb+1 and the output projection for block j-1 are
emitted interleaved into block j's attention chunk stream so the PE never
idles while the activation engine works through the exps.  PSUM: 4 banks
score/proj rotation + 2 banks out-proj/denominator rotation + 2 banks attnV
accumulators = 8.
"""

import math
from contextlib import ExitStack

import ml_dtypes
import numpy as np


import concourse.bass as bass
import concourse.mybir as mybir
from concourse import bacc
import concourse.tile as tile
from concourse.bass_utils import run_bass_kernel_spmd

F32 = mybir.dt.float32
BF16 = mybir.dt.bfloat16
EXP = mybir.ActivationFunctionType.Exp

B, S, H = 2, 2048, 1024
NH, DK, DV = 16, 64, 64
NCORE = 8
NCH = H // 128          # 8 contraction chunks over H
NJ = S // 512           # 4 query blocks of 512
NKC = S // 128          # 16 key chunks
NPAIR = 2               # head pairs per core
SCALE = 1.0 / math.sqrt(DK)
NEG_BIAS = -30000.0     # exp(x + NEG_BIAS) == 0.0 in fp32 for any real score
NWUP = 10               # dummy warm-up matmuls (HAM un-throttle)
_WARMUP = True          # dummy HAM warm-up matmuls

# consts layout (columns of the "smalls" f32 tensor)
_BQ0, _BK0, _NB0 = 0, 2, 4
_SMALLW = _NB0 + NKC    # bq(2) bk(2) nbias(16)
_SWMDW = 128 + 256      # swap(128) mdiag(2*128), bf16


def _emit(nc, d):
    """Emit the per-core program.  d maps names -> DRAM tensor handles."""
    with tile.TileContext(nc) as tc, ExitStack() as top:
        consts = top.enter_context(tc.tile_pool(name="consts", bufs=1))
        persist = top.enter_context(tc.tile_pool(name="persist", bufs=1))

        # ---- tiles for constants / weights ----
        smalls = consts.tile([128, _SMALLW], F32, tag="smalls", name="smalls")
        bq_sb = smalls[:, _BQ0:_BQ0 + 2]
        bk_sb = smalls[:, _BK0:_BK0 + 2]
        nbias_sb = smalls[:, _NB0:_NB0 + NKC]
        swapmd = consts.tile([128, _SWMDW], BF16, tag="swapmd", name="swapmd")
        swap_sb = swapmd[:, 0:128]
        mdiag_sb = swapmd[:, 128:384].rearrange("p (a b) -> p a b", a=2)
        wqq_sb = [consts.tile([128, NCH * 128], BF16, tag=f"wqq{p}",
                              name=f"wqq{p}sb") for p in range(NPAIR)]
        wkk_sb = [consts.tile([128, NCH * 128], BF16, tag=f"wkk{p}",
                              name=f"wkk{p}sb") for p in range(NPAIR)]
        wv_sb = consts.tile([128, NCH * 256], BF16, tag="wv", name="wvsb")
        wo_sb = consts.tile([128, 2 * 1024], BF16, tag="wo", name="wosb")
        # x^T pre-arranged on host: [128, block j, chunk c, 512 tokens]
        xt_sb = persist.tile([128, NJ, NCH, 512], BF16, tag="xt", name="xtsb")
        wup = consts.tile([128, 512], BF16, tag="wup", name="wupsb")

        # ---- PSUM pools: 4 (scores+proj) + 2 (outproj/den) + 2 (oA,oB) ----
        sp = top.enter_context(tc.tile_pool(name="sp", bufs=2, space="PSUM"))
        rot = top.enter_context(tc.tile_pool(name="rot", bufs=2, space="PSUM"))
        op = top.enter_context(tc.tile_pool(name="op", bufs=1, space="PSUM"))

        # ---- HAM warm-up: dummy matmuls on a zeroed tile, discarded ----
        if _WARMUP:
            nc.gpsimd.memset(wup, 0.0)
            for i in range(NWUP):
                pw = rot.tile([128, 512], F32, tag="r", name=f"wup{i}")
                nc.tensor.matmul(pw, wup[:, 0:128], wup, start=True, stop=True)

        # ---- input DMAs on the two HWDGE rings (sync + scalar).
        # Ordered by first consumption; ~0.6us dispatch each.
        xtd = d["xt"]
        BW = NCH * 512  # dram cols per block

        def xt_piece(eng, j, c0, c1):
            eng.dma_start(out=xt_sb[:, j, c0:c1, :],
                          in_=xtd[:, j * BW + c0 * 512: j * BW + c1 * 512])

        # Arrival order tracks consumption order: Q0/K0 weights first (the
        # interleaved chunk-major Q0K0 stream consumes them against xt
        # pieces as they land), then pair-1 weights, V, out-proj.
        nc.sync.dma_start(out=wqq_sb[0], in_=d["wqq"][0])
        xt_piece(nc.scalar, 0, 0, 2)
        nc.sync.dma_start(out=wkk_sb[0], in_=d["wkk"][0])
        xt_piece(nc.scalar, 0, 2, 4)
        nc.sync.dma_start(out=smalls, in_=d["smalls"][:])
        nc.sync.dma_start(out=wqq_sb[1], in_=d["wqq"][1])
        xt_piece(nc.scalar, 0, 4, 6)
        nc.sync.dma_start(out=wkk_sb[1], in_=d["wkk"][1])
        xt_piece(nc.scalar, 0, 6, 8)
        nc.sync.dma_start(out=wv_sb[:, 0:1024], in_=d["wv"][:, 0:1024])
        nc.sync.dma_start(out=wv_sb[:, 1024:2048], in_=d["wv"][:, 1024:2048])
        nc.sync.dma_start(out=swapmd, in_=d["swapmd"][:])
        # later blocks ride the sync ring: its engine only waits anyway,
        # while dispatch slots on the scalar ring would stall the ACT queue
        # mid-stream (each dma_start holds the engine ~0.6us).
        xt_piece(nc.sync, 1, 0, 4)
        xt_piece(nc.sync, 1, 4, 8)
        nc.sync.dma_start(out=wo_sb[:, 0:1024], in_=d["wo"][:, 0:1024])
        nc.sync.dma_start(out=wo_sb[:, 1024:2048], in_=d["wo"][:, 1024:2048])
        xt_piece(nc.sync, 2, 0, 4)
        xt_piece(nc.sync, 2, 4, 8)
        xt_piece(nc.sync, 3, 0, 4)
        xt_piece(nc.sync, 3, 4, 8)

        # ---- persistent activations ----
        qt_sb = []   # per pair: [128, S]; rows 0:64 head A Q^T, 64:128 head B
        kt_sb = []
        attnT = []   # per pair: [128, S]; normalized attn^T (dims on rows)
        for p in range(NPAIR):
            qt_sb.append(persist.tile([128, S], BF16, tag=f"qt{p}",
                                      name=f"qt{p}sb"))
            kt_sb.append(persist.tile([128, S], BF16, tag=f"kt{p}",
                                      name=f"kt{p}sb"))
            attnT.append(persist.tile([128, S], BF16, tag=f"at{p}",
                                      name=f"at{p}sb"))
        # V_aug [128, parity, pair, keycols]: even heads (A) = [V | ones],
        # odd heads (B) = [ones | V]
        vaug = persist.tile([128, 2, 2, NKC * 128], BF16, tag="vaug",
                            name="vaugsb")
        nc.gpsimd.memset(vaug, 1.0)

        # ---- SBUF work pools ----
        ptp = top.enter_context(tc.tile_pool(name="ptp", bufs=6))
        nrm = top.enter_context(tc.tile_pool(name="nrm", bufs=3))
        ysb = top.enter_context(tc.tile_pool(name="ysb", bufs=3))

        def qk_interleaved(jb, p):
            """Q and K projections for (pair p, block jb) with the two
            accumulation streams interleaved chunk-major: 4 matmuls become
            ready per arriving 2-chunk xt DMA piece, keeping the PE
            continuously busy through the DMA-paced prologue."""
            jsl = slice(jb * 512, (jb + 1) * 512)
            psq = sp.tile([128, 2, 512], F32, tag="s", name=f"psqi{p}{jb}")
            psk = sp.tile([128, 2, 512], F32, tag="s", name=f"pski{p}{jb}")
            for c in range(NCH):
                nc.tensor.matmul(psq[:, 0, :], wqq_sb[p][:, c * 128:(c + 1) * 128],
                                 xt_sb[:, jb, c, :], start=(c == 0),
                                 stop=(c == NCH - 1), skip_group_check=True)
                nc.tensor.matmul(psk[:, 0, :], wkk_sb[p][:, c * 128:(c + 1) * 128],
                                 xt_sb[:, jb, c, :], start=(c == 0),
                                 stop=(c == NCH - 1), skip_group_check=True)
            nc.vector.tensor_scalar_add(qt_sb[p][:, jsl], psq[:, 0, :],
                                        bq_sb[:, p:p + 1])
            nc.vector.tensor_scalar_add(kt_sb[p][:, jsl], psk[:, 0, :],
                                        bk_sb[:, p:p + 1])

        def proj_units(jb, kinds="qkv", pairs=None):
            """Q/K/V projection for query block jb: independent units."""
            jsl = slice(jb * 512, (jb + 1) * 512)
            units = []
            for p in (range(NPAIR) if pairs is None else pairs):
                for wsb, bsb, dst, nm in (
                    (wqq_sb[p], bq_sb, qt_sb[p], "q"),
                    (wkk_sb[p], bk_sb, kt_sb[p], "k"),
                ):
                    if nm not in kinds:
                        continue
                    def u(p=p, wsb=wsb, bsb=bsb, dst=dst, nm=nm, jsl=jsl, jb=jb):
                        ps = sp.tile([128, 2, 512], F32, tag="s",
                                     name=f"ps{nm}{p}{jb}")
                        for c in range(NCH):
                            nc.tensor.matmul(
                                ps[:, 0, :],
                                wsb[:, c * 128:(c + 1) * 128],
                                xt_sb[:, jb, c, :],
                                start=(c == 0), stop=(c == NCH - 1),
                            )
                        nc.vector.tensor_scalar_add(dst[:, jsl], ps[:, 0, :],
                                                    bsb[:, p:p + 1])
                    units.append(u)
            if "v" not in kinds:
                return units
            for t in range(4 * jb, 4 * jb + 4):
                def u(t=t, jb=jb):
                    tl = t - 4 * jb
                    ps = sp.tile([128, 4, 128], F32, tag="s", name=f"psv{t}")
                    for c in range(NCH):
                        nc.tensor.matmul(
                            ps[:, 0:2, :],
                            xt_sb[:, jb, c, tl * 128:(tl + 1) * 128],
                            wv_sb[:, c * 256:(c + 1) * 256],
                            start=(c == 0), stop=(c == NCH - 1),
                        )
                    # ps cols = [h0|h1|h2|h3] x 64; even heads' V to parity 0
                    # front half, odd heads' V to parity 1 back half.
                    nc.vector.tensor_copy(
                        vaug[:, 0, :, t * 128:t * 128 + 64], ps[:, 0:2, 0:64])
                    nc.vector.tensor_copy(
                        vaug[:, 1, :, t * 128 + 64:(t + 1) * 128],
                        ps[:, 0:2, 64:128])
                units.append(u)
            return units

        def psf_units(j, tail=False, qs=None):
            """Output projection for query block j: 8 independent units."""
            units = []
            for q in (range(4 * j, 4 * j + 4) if qs is None else qs):
                yt = ysb.tile([128, 1024], BF16, tag="y", name=f"yt{q}")
                for half in range(2):
                    def u(q=q, half=half, tail=tail, yt=yt):
                        pf = rot.tile([128, 512], F32, tag="r",
                                      name=f"pf{q}{half}")
                        for p in range(NPAIR):
                            nc.tensor.matmul(
                                pf,
                                attnT[p][:, q * 128:(q + 1) * 128],
                                wo_sb[:, p * 1024 + half * 512:
                                      p * 1024 + half * 512 + 512],
                                start=(p == 0), stop=(p == 1),
                            )
                        ysl = slice(half * 512, (half + 1) * 512)
                        if tail:
                            eng = nc.vector if half == 0 else nc.scalar
                            if eng is nc.scalar:
                                nc.scalar.copy(yt[:, ysl], pf)
                            else:
                                nc.vector.tensor_copy(yt[:, ysl], pf)
                            if q == S // 128 - 1:
                                # final q: ship each half immediately (sync
                                # ring only -- a scalar-ring dispatch would
                                # stall the ACT copy queue)
                                nc.sync.dma_start(
                                    out=d["y"][q * 128:(q + 1) * 128, ysl],
                                    in_=yt[:, ysl])
                            elif half == 1:
                                deng = nc.sync if q % 2 else nc.gpsimd
                                deng.dma_start(
                                    out=d["y"][q * 128:(q + 1) * 128, :],
                                    in_=yt)
                        else:
                            nc.vector.tensor_copy(yt[:, ysl], pf)
                            if half == 1:
                                nc.gpsimd.dma_start(
                                    out=d["y"][q * 128:(q + 1) * 128, :],
                                    in_=yt)
                    units.append(u)
            return units

        def emit_scores(p, j, c):
            """Scores + exp (+ diag mask) for chunk c; returns attnV args.

            The two heads' matmuls are independent 64x128 row tiles (T0 and
            T8, auto-derived from the operands' base partitions) and execute
            concurrently on the PE array.
            """
            t = c - 4 * j
            fo = 128 * t if t > 0 else 0
            w = 512 - fo
            qsl = slice(j * 512 + fo, (j + 1) * 512)
            with tc.high_priority(offset=165):
                s2 = sp.tile([128, 2, 512], F32, tag="s", name=f"s{p}{j}{c}")
                # The two heads' matmuls are independent 64x128 row tiles
                # (T0/T8) and execute CONCURRENTLY on the PE (~3ns apart).
                # Do NOT be tempted to also col-split them into 64x64
                # quadrants: column-tiling mode interleaved with 128-col
                # matmuls hangs TRN2.
                nc.tensor.matmul(s2[:, 0:1, :w],
                                 kt_sb[p][0:64, c * 128:(c + 1) * 128],
                                 qt_sb[p][0:64, qsl],
                                 start=True, stop=True)
                nc.tensor.matmul(s2[:, 1:2, :w],
                                 kt_sb[p][64:128, c * 128:(c + 1) * 128],
                                 qt_sb[p][64:128, qsl],
                                 start=True, stop=True)
                p2 = ptp.tile([128, 2, 512], BF16, tag="p", name=f"p{p}{j}{c}")
                nc.scalar.activation(p2[:, :, :w], s2[:, :, :w], EXP,
                                     bias=nbias_sb[:, c:c + 1], scale=SCALE)
            if t >= 0:
                # diagonal 128x128 block: zero keys below the diagonal for
                # both heads in one op.  High priority: the chunk's attnV
                # waits on this through bursty DVE queues.
                with tc.high_priority(offset=165):
                    nc.vector.tensor_mul(p2[:, :, 0:128], p2[:, :, 0:128],
                                         mdiag_sb)
            return p2, fo, w

        def emit_attnv(p, j, c, oA, oB, cmax, p2, fo, w):
            ksl = slice(c * 128, (c + 1) * 128)
            nc.tensor.matmul(oA[:, fo:512], vaug[:, 0, p, ksl],
                             p2[:, 0:1, :w], start=(c == 0), stop=(c == cmax))
            nc.tensor.matmul(oB[:, fo:512], vaug[:, 1, p, ksl],
                             p2[:, 1:2, :w], start=(c == 0), stop=(c == cmax))

        def emit_norm(p, j, oA, oB, halves=1, after_half=None):
            # denA = oA[64:128], denB = oB[0:64]; swap halves on the PE so
            # each reciprocal lands on its numerator's partitions.  High
            # priority so the scr copies jump the DVE queue (the PE's den
            # matmul and the next pair's accumulator reuse both wait on
            # this path).
            with tc.high_priority(offset=165):
                scr = nrm.tile([128, 512], BF16, tag="scr", name=f"scr{p}{j}")
                nc.vector.tensor_copy(scr[64:128, :], oA[64:128, :])
                nc.vector.tensor_copy(scr[0:64, :], oB[0:64, :])
                den2 = rot.tile([128, 512], F32, tag="r", name=f"den{p}{j}")
                nc.tensor.matmul(den2, swap_sb, scr, start=True, stop=True)
                rec = nrm.tile([128, 512], F32, tag="rec", name=f"rec{p}{j}")
                nc.vector.reciprocal_approx_fast(out=rec, in_=den2)
            for h in range(halves):
                hw = 512 // halves
                csl = slice(j * 512 + h * hw, j * 512 + (h + 1) * hw)
                osl = slice(h * hw, (h + 1) * hw)
                nc.vector.tensor_mul(attnT[p][0:64, csl], oA[0:64, osl],
                                     rec[0:64, osl])
                nc.vector.tensor_mul(attnT[p][64:128, csl], oB[64:128, osl],
                                     rec[64:128, osl])
                if after_half is not None:
                    after_half(h)

        # ---- main schedule ----
        qk_interleaved(0, 0)
        for u in proj_units(0, kinds="qk", pairs=[1]) + proj_units(0, kinds="v"):
            u()
        # Filler assignment keeps every step PE-bound.  Block 3's K and V
        # projections are only consumed from chunk 12 of step 3, so they
        # slide into step 3 itself as guaranteed-ready PE filler for its
        # exp-heavy stretch; out-projections lag two steps for the same
        # reason.
        for j in range(NJ):
            if j == 0:
                fillers = proj_units(1)
            elif j == 1:
                fillers = proj_units(2) + psf_units(0)
            elif j == 2:
                fillers = proj_units(3, kinds="q") + psf_units(1)
            else:
                # K3/V3 lead: they are consumed by this step's own chunks
                # from c=12.
                fillers = proj_units(3, kinds="kv") + psf_units(2)
            nch_j = 4 * j + 4
            total_chunks = 2 * nch_j
            # Finish fillers a few chunks early so their PSUM drains don't
            # jam the DVE right when the tail norm path needs it.
            denom = total_chunks if j == 0 else max(1, total_chunks - 4)
            done = 0
            ci = 0
            for p in range(NPAIR):
                oA = op.tile([128, 512], F32, tag="oA", name=f"oA{p}{j}")
                oB = op.tile([128, 512], F32, tag="oB", name=f"oB{p}{j}")
                pend = []
                for c in range(nch_j):
                    pend.append((c,) + emit_scores(p, j, c))
                    if len(pend) > 3:
                        c0, p2, fo, w = pend.pop(0)
                        emit_attnv(p, j, c0, oA, oB, nch_j - 1, p2, fo, w)
                    ci += 1
                    want = min(len(fillers), ci * len(fillers) // denom)
                    while done < want:
                        fillers[done]()
                        done += 1
                for c0, p2, fo, w in pend:
                    emit_attnv(p, j, c0, oA, oB, nch_j - 1, p2, fo, w)
                if j == NJ - 1 and p == NPAIR - 1:
                    # keep the PE (and its HAM clock) busy through the
                    # norm chain's DVE round-trip so the tail out-proj
                    # matmuls run at 2.4 GHz
                    for i in range(4):
                        pw = rot.tile([128, 512], F32, tag="r",
                                      name=f"tailwup{i}")
                        nc.tensor.matmul(pw, wup[:, 0:128], wup,
                                         start=True, stop=True)

                    def tail_half(h, j=j):
                        for u in psf_units(j, tail=True, qs=(4 * j + h,)):
                            u()
                    emit_norm(p, j, oA, oB, halves=4, after_half=tail_half)
                else:
                    # two filler matmuls cover the den matmul's wait on the
                    # scr copies' DVE round-trip at every pair boundary
                    for i in range(2):
                        pw = rot.tile([128, 512], F32, tag="r",
                                      name=f"bwup{p}{j}{i}")
                        nc.tensor.matmul(pw, wup[:, 0:128], wup,
                                         start=True, stop=True)
                    emit_norm(p, j, oA, oB)
            while done < len(fillers):
                fillers[done]()
                done += 1

        if _DEBUG:
            for p in range(NPAIR):
                nc.sync.dma_start(out=d[f"dbg_qt{p}"][:], in_=qt_sb[p].bitcast(F32))
                nc.sync.dma_start(out=d[f"dbg_kt{p}"][:], in_=kt_sb[p].bitcast(F32))
                nc.sync.dma_start(out=d[f"dbg_at{p}"][:], in_=attnT[p].bitcast(F32))


_NC_CACHE = {}
_DEBUG = False


def _get_nc():
    key = "nc"
    if key not in _NC_CACHE:
        nc = bacc.Bacc(None, target_bir_lowering=False)
        d = {
            "xt": nc.dram_tensor("xt", [128, NJ * NCH * 512], BF16,
                                 kind="ExternalInput"),
            "wqq": nc.dram_tensor("wqq", [NPAIR, 128, NCH * 128], BF16,
                                  kind="ExternalInput"),
            "wkk": nc.dram_tensor("wkk", [NPAIR, 128, NCH * 128], BF16,
                                  kind="ExternalInput"),
            "wv": nc.dram_tensor("wv", [128, NCH * 256], BF16, kind="ExternalInput"),
            "wo": nc.dram_tensor("wo", [128, 2 * 1024], BF16, kind="ExternalInput"),
            "smalls": nc.dram_tensor("smalls", [128, _SMALLW], F32,
                                     kind="ExternalInput"),
            "swapmd": nc.dram_tensor("swapmd", [128, _SWMDW], BF16,
                                     kind="ExternalInput"),
            "y": nc.dram_tensor("y", [S, H], BF16, kind="ExternalOutput"),
        }
        if _DEBUG:
            for p in range(NPAIR):
                d[f"dbg_qt{p}"] = nc.dram_tensor(f"dbg_qt{p}", [128, S], F32,
                                                 kind="ExternalOutput")
                d[f"dbg_kt{p}"] = nc.dram_tensor(f"dbg_kt{p}", [128, S], F32,
                                                 kind="ExternalOutput")
                d[f"dbg_at{p}"] = nc.dram_tensor(f"dbg_at{p}", [128, S], F32,
                                                 kind="ExternalOutput")
        _emit(nc, d)
        nc.finalize()
        _NC_CACHE[key] = nc
    return _NC_CACHE[key]


def _chunked(w, ncols):
    """[H, ncols] -> [128, NCH*ncols] with chunk c of rows at cols c*ncols."""
    return np.ascontiguousarray(
        w.reshape(NCH, 128, ncols).transpose(1, 0, 2).reshape(128, NCH * ncols))


def _make_in_maps(batch, input_ids, W_Q, W_K, W_V, W_O, b_Q, b_K):
    m = np.triu(np.ones((128, 128), np.float32))
    mdiag2 = np.stack([m, m], axis=1).reshape(128, 256)  # [128, 2*128]
    swap = np.zeros((128, 128), np.float32)
    swap[64:128, 0:64] = np.eye(64, dtype=np.float32)
    swap[0:64, 64:128] = np.eye(64, dtype=np.float32)
    bf = ml_dtypes.bfloat16
    in_maps = []
    for core in range(NCORE):
        b, g = divmod(core, 4)
        base = 256 * g  # first feature column of this core's 4 heads
        wqq = np.stack([_chunked(W_Q[:, base + 128 * p: base + 128 * (p + 1)], 128)
                        for p in range(NPAIR)])
        wkk = np.stack([_chunked(W_K[:, base + 128 * p: base + 128 * (p + 1)], 128)
                        for p in range(NPAIR)])
        wv = _chunked(W_V[:, base: base + 256], 256)
        wo = np.ascontiguousarray(
            W_O[base: base + 256, :].reshape(2, 128, H)
            .transpose(1, 0, 2).reshape(128, 2 * H))
        bq = np.stack([b_Q[base + 128 * p: base + 128 * (p + 1)]
                       for p in range(NPAIR)], axis=1)
        bk = np.stack([b_K[base + 128 * p: base + 128 * (p + 1)]
                       for p in range(NPAIR)], axis=1)
        keep = input_ids[b] != 0
        nbias = np.where(keep, 0.0, NEG_BIAS).astype(np.float32)
        nbias = nbias.reshape(NKC, 128).T
        smalls = np.concatenate([bq, bk, nbias], axis=1)
        assert smalls.shape == (128, _SMALLW)
        swapmd = np.concatenate([swap, mdiag2], axis=1).astype(bf)
        assert swapmd.shape == (128, _SWMDW)
        # x^T block-major: [128, block j, chunk c, 512]
        xt = batch[b].T.reshape(NCH, 128, NJ, 512).transpose(1, 2, 0, 3)
        xt = np.ascontiguousarray(xt).reshape(128, NJ * NCH * 512)
        in_maps.append({
            "xt": xt.astype(bf), "wqq": wqq.astype(bf),
            "wkk": wkk.astype(bf), "wv": wv.astype(bf),
            "wo": wo.astype(bf),
            "smalls": np.ascontiguousarray(smalls.astype(np.float32)),
            "swapmd": np.ascontiguousarray(swapmd),
        })
    return in_maps


def _run(in_maps, **kwargs):
    nc = _get_nc()
    return run_bass_kernel_spmd(nc, in_maps, core_ids=list(range(NCORE)), **kwargs)


def kernel(batch, input_ids, W_Q, W_K, W_V, b_Q, b_K, b_V, W_O, b_O,
           _results_out=None, **run_kwargs):
    batch = np.asarray(batch, np.float32)
    input_ids = np.asarray(input_ids)
    W_Q, W_K, W_V = (np.asarray(a, np.float32) for a in (W_Q, W_K, W_V))
    b_Q, b_K, b_V = (np.asarray(a, np.float32) for a in (b_Q, b_K, b_V))
    W_O = np.asarray(W_O, np.float32)
    b_O = np.asarray(b_O, np.float32)

    in_maps = _make_in_maps(batch, input_ids, W_Q, W_K, W_V, W_O, b_Q, b_K)
    res = _run(in_maps, **run_kwargs)
    if _results_out is not None:
        _results_out.append(res)
    ys = [np.asarray(res.results[c]["y"], np.float32) for c in range(NCORE)]
    out = np.stack([sum(ys[4 * b: 4 * b + 4]) for b in range(B)], axis=0)
    # b_V enters as attn@1 * b_V = b_V (softmax rows sum to 1), then @ W_O.
    const_row = (b_V @ W_O + b_O).astype(np.float32)
    return (out + const_row).astype(np.float32)


# revision 64
# speedup vs baseline: 1.0091x; 1.0091x over previous
"""Multi-head attention (B=2, S=2048, H=1024, NH=16, DK=DV=64) on 8 TRN2 cores.

Sharding: data-parallel over batch (2 groups of 4 cores) x tensor-parallel
over heads (4 heads per core, as 2 pairs of 2).  Each core computes, for its
batch sample and its 4 heads:

    Q^T/K^T projections (features on partitions), V projection (natural),
    S^T = K @ Q^T per 128-key chunk (causal chunks only); the two heads'
    matmuls are independent 64x128 row tiles (T0/T8) that execute
    concurrently on the PE array (~3ns start offset),
    P^T = exp(S^T/8 + pad_bias)   (one fused exp over both heads; no
    max-subtraction needed: |scores| ~ N(0,1)),
    out^T = V_aug^T @ P^T  where V_aug = [V | ones] for head A and
    [ones | V] for head B, so oA = [attnA; denA] and oB = [denB; attnB].
    A single PE matmul with a half-swap permutation aligns both
    denominators with their numerators; one reciprocal + two muls
    normalize directly into attnT (bf16).
    y_partial = attnT^T @ W_O_rows   (row-sharded W_O, all bf16).

Host sums the 4 partials per batch and adds (b_V @ W_O + b_O) (exact since
softmax rows sum to 1).

Pipelining: x^T is uploaded host-pre-arranged as [128, NJ, NCH, 512] so each
query block is one contiguous 1MB DMA region; a handful of dummy warm-up
matmuls at program start bring the PE HAM clock to 2.4 GHz before real work;
projections for query block jb+1 and the output projection for block j-1 are
emitted interleaved into block j's attention chunk stream so the PE never
idles while the activation engine works through the exps.  PSUM: 4 banks
score/proj rotation + 2 banks out-proj/denominator rotation + 2 banks attnV
accumulators = 8.
"""

import math
from contextlib import ExitStack

import ml_dtypes
import numpy as np


import concourse.bass as bass
import concourse.mybir as mybir
from concourse import bacc
import concourse.tile as tile
from concourse.bass_utils import run_bass_kernel_spmd

F32 = mybir.dt.float32
BF16 = mybir.dt.bfloat16
EXP = mybir.ActivationFunctionType.Exp

B, S, H = 2, 2048, 1024
NH, DK, DV = 16, 64, 64
NCORE = 8
NCH = H // 128          # 8 contraction chunks over H
NJ = S // 512           # 4 query blocks of 512
NKC = S // 128          # 16 key chunks
NPAIR = 2               # head pairs per core
SCALE = 1.0 / math.sqrt(DK)
NEG_BIAS = -30000.0     # exp(x + NEG_BIAS) == 0.0 in fp32 for any real score
NWUP = 10               # dummy warm-up matmuls (HAM un-throttle)
_WARMUP = True          # dummy HAM warm-up matmuls

# consts layout (columns of the "smalls" f32 tensor)
_BQ0, _BK0, _NB0 = 0, 2, 4
_SMALLW = _NB0 + NKC    # bq(2) bk(2) nbias(16)
_SWMDW = 128 + 256      # swap(128) mdiag(2*128), bf16


def _emit(nc, d):
    """Emit the per-core program.  d maps names -> DRAM tensor handles."""
    with tile.TileContext(nc) as tc, ExitStack() as top:
        consts = top.enter_context(tc.tile_pool(name="consts", bufs=1))
        persist = top.enter_context(tc.tile_pool(name="persist", bufs=1))

        # ---- tiles for constants / weights ----
        smalls = consts.tile([128, _SMALLW], F32, tag="smalls", name="smalls")
        bq_sb = smalls[:, _BQ0:_BQ0 + 2]
        bk_sb = smalls[:, _BK0:_BK0 + 2]
        nbias_sb = smalls[:, _NB0:_NB0 + NKC]
        swapmd = consts.tile([128, _SWMDW], BF16, tag="swapmd", name="swapmd")
        swap_sb = swapmd[:, 0:128]
        mdiag_sb = swapmd[:, 128:384].rearrange("p (a b) -> p a b", a=2)
        wqq_sb = [consts.tile([128, NCH * 128], BF16, tag=f"wqq{p}",
                              name=f"wqq{p}sb") for p in range(NPAIR)]
        wkk_sb = [consts.tile([128, NCH * 128], BF16, tag=f"wkk{p}",
                              name=f"wkk{p}sb") for p in range(NPAIR)]
        wv_sb = consts.tile([128, NCH * 256], BF16, tag="wv", name="wvsb")
        wo_sb = consts.tile([128, 2 * 1024], BF16, tag="wo", name="wosb")
        # x^T pre-arranged on host: [128, block j, chunk c, 512 tokens]
        xt_sb = persist.tile([128, NJ, NCH, 512], BF16, tag="xt", name="xtsb")
        wup = consts.tile([128, 512], BF16, tag="wup", name="wupsb")

        # ---- PSUM pools: 4 (scores+proj) + 2 (outproj/den) + 2 (oA,oB) ----
        sp = top.enter_context(tc.tile_pool(name="sp", bufs=2, space="PSUM"))
        rot = top.enter_context(tc.tile_pool(name="rot", bufs=2, space="PSUM"))
        op = top.enter_context(tc.tile_pool(name="op", bufs=1, space="PSUM"))

        # ---- HAM warm-up: dummy matmuls on a zeroed tile, discarded ----
        if _WARMUP:
            nc.gpsimd.memset(wup, 0.0)
            for i in range(NWUP):
                pw = rot.tile([128, 512], F32, tag="r", name=f"wup{i}")
                nc.tensor.matmul(pw, wup[:, 0:128], wup, start=True, stop=True)

        # ---- input DMAs on the two HWDGE rings (sync + scalar).
        # Ordered by first consumption; ~0.6us dispatch each.
        xtd = d["xt"]
        BW = NCH * 512  # dram cols per block

        def xt_piece(eng, j, c0, c1):
            eng.dma_start(out=xt_sb[:, j, c0:c1, :],
                          in_=xtd[:, j * BW + c0 * 512: j * BW + c1 * 512])

        # Arrival order tracks consumption order: Q0/K0 weights first (the
        # interleaved chunk-major Q0K0 stream consumes them against xt
        # pieces as they land), then pair-1 weights, V, out-proj.
        nc.sync.dma_start(out=wqq_sb[0], in_=d["wqq"][0])
        xt_piece(nc.scalar, 0, 0, 2)
        nc.sync.dma_start(out=wkk_sb[0], in_=d["wkk"][0])
        xt_piece(nc.scalar, 0, 2, 4)
        nc.sync.dma_start(out=smalls, in_=d["smalls"][:])
        nc.sync.dma_start(out=wqq_sb[1], in_=d["wqq"][1])
        xt_piece(nc.scalar, 0, 4, 6)
        nc.sync.dma_start(out=wkk_sb[1], in_=d["wkk"][1])
        xt_piece(nc.scalar, 0, 6, 8)
        nc.sync.dma_start(out=wv_sb[:, 0:1024], in_=d["wv"][:, 0:1024])
        nc.sync.dma_start(out=wv_sb[:, 1024:2048], in_=d["wv"][:, 1024:2048])
        nc.sync.dma_start(out=swapmd, in_=d["swapmd"][:])
        # later blocks ride the sync ring: its engine only waits anyway,
        # while dispatch slots on the scalar ring would stall the ACT queue
        # mid-stream (each dma_start holds the engine ~0.6us).
        xt_piece(nc.sync, 1, 0, 4)
        xt_piece(nc.sync, 1, 4, 8)
        nc.sync.dma_start(out=wo_sb[:, 0:1024], in_=d["wo"][:, 0:1024])
        nc.sync.dma_start(out=wo_sb[:, 1024:2048], in_=d["wo"][:, 1024:2048])
        xt_piece(nc.sync, 2, 0, 4)
        xt_piece(nc.sync, 2, 4, 8)
        xt_piece(nc.sync, 3, 0, 4)
        xt_piece(nc.sync, 3, 4, 8)

        # ---- persistent activations ----
        qt_sb = []   # per pair: [128, S]; rows 0:64 head A Q^T, 64:128 head B
        kt_sb = []
        attnT = []   # per pair: [128, S]; normalized attn^T (dims on rows)
        for p in range(NPAIR):
            qt_sb.append(persist.tile([128, S], BF16, tag=f"qt{p}",
                                      name=f"qt{p}sb"))
            kt_sb.append(persist.tile([128, S], BF16, tag=f"kt{p}",
                                      name=f"kt{p}sb"))
            attnT.append(persist.tile([128, S], BF16, tag=f"at{p}",
                                      name=f"at{p}sb"))
        # V_aug [128, parity, pair, keycols]: even heads (A) = [V | ones],
        # odd heads (B) = [ones | V]
        vaug = persist.tile([128, 2, 2, NKC * 128], BF16, tag="vaug",
                            name="vaugsb")
        nc.gpsimd.memset(vaug, 1.0)

        # ---- SBUF work pools ----
        ptp = top.enter_context(tc.tile_pool(name="ptp", bufs=6))
        nrm = top.enter_context(tc.tile_pool(name="nrm", bufs=3))
        ysb = top.enter_context(tc.tile_pool(name="ysb", bufs=3))

        def qk_interleaved(jb, p):
            """Q and K projections for (pair p, block jb) with the two
            accumulation streams interleaved chunk-major: 4 matmuls become
            ready per arriving 2-chunk xt DMA piece, keeping the PE
            continuously busy through the DMA-paced prologue."""
            jsl = slice(jb * 512, (jb + 1) * 512)
            psq = sp.tile([128, 2, 512], F32, tag="s", name=f"psqi{p}{jb}")
            psk = sp.tile([128, 2, 512], F32, tag="s", name=f"pski{p}{jb}")
            for c in range(NCH):
                nc.tensor.matmul(psq[:, 0, :], wqq_sb[p][:, c * 128:(c + 1) * 128],
                                 xt_sb[:, jb, c, :], start=(c == 0),
                                 stop=(c == NCH - 1), skip_group_check=True)
                nc.tensor.matmul(psk[:, 0, :], wkk_sb[p][:, c * 128:(c + 1) * 128],
                                 xt_sb[:, jb, c, :], start=(c == 0),
                                 stop=(c == NCH - 1), skip_group_check=True)
            nc.vector.tensor_scalar_add(qt_sb[p][:, jsl], psq[:, 0, :],
                                        bq_sb[:, p:p + 1])
            nc.vector.tensor_scalar_add(kt_sb[p][:, jsl], psk[:, 0, :],
                                        bk_sb[:, p:p + 1])

        def proj_units(jb, kinds="qkv", pairs=None):
            """Q/K/V projection for query block jb: independent units."""
            jsl = slice(jb * 512, (jb + 1) * 512)
            units = []
            for p in (range(NPAIR) if pairs is None else pairs):
                for wsb, bsb, dst, nm in (
                    (wqq_sb[p], bq_sb, qt_sb[p], "q"),
                    (wkk_sb[p], bk_sb, kt_sb[p], "k"),
                ):
                    if nm not in kinds:
                        continue
                    def u(p=p, wsb=wsb, bsb=bsb, dst=dst, nm=nm, jsl=jsl, jb=jb):
                        ps = sp.tile([128, 2, 512], F32, tag="s",
                                     name=f"ps{nm}{p}{jb}")
                        for c in range(NCH):
                            nc.tensor.matmul(
                                ps[:, 0, :],
                                wsb[:, c * 128:(c + 1) * 128],
                                xt_sb[:, jb, c, :],
                                start=(c == 0), stop=(c == NCH - 1),
                            )
                        nc.vector.tensor_scalar_add(dst[:, jsl], ps[:, 0, :],
                                                    bsb[:, p:p + 1])
                    units.append(u)
            if "v" not in kinds:
                return units
            for t in range(4 * jb, 4 * jb + 4):
                def u(t=t, jb=jb):
                    tl = t - 4 * jb
                    ps = sp.tile([128, 4, 128], F32, tag="s", name=f"psv{t}")
                    for c in range(NCH):
                        nc.tensor.matmul(
                            ps[:, 0:2, :],
                            xt_sb[:, jb, c, tl * 128:(tl + 1) * 128],
                            wv_sb[:, c * 256:(c + 1) * 256],
                            start=(c == 0), stop=(c == NCH - 1),
                        )
                    # ps cols = [h0|h1|h2|h3] x 64; even heads' V to parity 0
                    # front half, odd heads' V to parity 1 back half.
                    nc.vector.tensor_copy(
                        vaug[:, 0, :, t * 128:t * 128 + 64], ps[:, 0:2, 0:64])
                    nc.vector.tensor_copy(
                        vaug[:, 1, :, t * 128 + 64:(t + 1) * 128],
                        ps[:, 0:2, 64:128])
                units.append(u)
            return units

        def psf_units(j, tail=False, qs=None):
            """Output projection for query block j: 8 independent units."""
            units = []
            for q in (range(4 * j, 4 * j + 4) if qs is None else qs):
                yt = ysb.tile([128, 1024], BF16, tag="y", name=f"yt{q}")
                for half in range(2):
                    def u(q=q, half=half, tail=tail, yt=yt):
                        pf = rot.tile([128, 512], F32, tag="r",
                                      name=f"pf{q}{half}")
                        for p in range(NPAIR):
                            nc.tensor.matmul(
                                pf,
                                attnT[p][:, q * 128:(q + 1) * 128],
                                wo_sb[:, p * 1024 + half * 512:
                                      p * 1024 + half * 512 + 512],
                                start=(p == 0), stop=(p == 1),
                            )
                        ysl = slice(half * 512, (half + 1) * 512)
                        if tail:
                            eng = nc.vector if half == 0 else nc.scalar
                            if eng is nc.scalar:
                                nc.scalar.copy(yt[:, ysl], pf)
                            else:
                                nc.vector.tensor_copy(yt[:, ysl], pf)
                            if q == S // 128 - 1:
                                # final q: ship each half immediately (sync
                                # ring only -- a scalar-ring dispatch would
                                # stall the ACT copy queue)
                                nc.sync.dma_start(
                                    out=d["y"][q * 128:(q + 1) * 128, ysl],
                                    in_=yt[:, ysl])
                            elif half == 1:
                                deng = nc.sync if q % 2 else nc.gpsimd
                                deng.dma_start(
                                    out=d["y"][q * 128:(q + 1) * 128, :],
                                    in_=yt)
                        else:
                            nc.vector.tensor_copy(yt[:, ysl], pf)
                            if half == 1:
                                nc.gpsimd.dma_start(
                                    out=d["y"][q * 128:(q + 1) * 128, :],
                                    in_=yt)
                    units.append(u)
            return units

        def emit_scores(p, j, c):
            """Scores + exp (+ diag mask) for chunk c; returns attnV args.

            The two heads' matmuls are independent 64x128 row tiles (T0 and
            T8, auto-derived from the operands' base partitions) and execute
            concurrently on the PE array.
            """
            t = c - 4 * j
            fo = 128 * t if t > 0 else 0
            w = 512 - fo
            qsl = slice(j * 512 + fo, (j + 1) * 512)
            with tc.high_priority(offset=165):
                s2 = sp.tile([128, 2, 512], F32, tag="s", name=f"s{p}{j}{c}")
                # The two heads' matmuls are independent 64x128 row tiles
                # (T0/T8) and execute CONCURRENTLY on the PE (~3ns apart).
                # Do NOT be tempted to also col-split them into 64x64
                # quadrants: column-tiling mode interleaved with 128-col
                # matmuls hangs TRN2.
                nc.tensor.matmul(s2[:, 0:1, :w],
                                 kt_sb[p][0:64, c * 128:(c + 1) * 128],
                                 qt_sb[p][0:64, qsl],
                                 start=True, stop=True)
                nc.tensor.matmul(s2[:, 1:2, :w],
                                 kt_sb[p][64:128, c * 128:(c + 1) * 128],
                                 qt_sb[p][64:128, qsl],
                                 start=True, stop=True)
                p2 = ptp.tile([128, 2, 512], BF16, tag="p", name=f"p{p}{j}{c}")
                nc.scalar.activation(p2[:, :, :w], s2[:, :, :w], EXP,
                                     bias=nbias_sb[:, c:c + 1], scale=SCALE)
            if t >= 0:
                # diagonal 128x128 block: zero keys below the diagonal for
                # both heads in one op.  High priority: the chunk's attnV
                # waits on this through bursty DVE queues.
                with tc.high_priority(offset=165):
                    nc.vector.tensor_mul(p2[:, :, 0:128], p2[:, :, 0:128],
                                         mdiag_sb)
            return p2, fo, w

        def emit_attnv(p, j, c, oA, oB, cmax, p2, fo, w):
            ksl = slice(c * 128, (c + 1) * 128)
            nc.tensor.matmul(oA[:, fo:512], vaug[:, 0, p, ksl],
                             p2[:, 0:1, :w], start=(c == 0), stop=(c == cmax))
            nc.tensor.matmul(oB[:, fo:512], vaug[:, 1, p, ksl],
                             p2[:, 1:2, :w], start=(c == 0), stop=(c == cmax))

        def emit_norm(p, j, oA, oB, halves=1, after_half=None):
            # denA = oA[64:128], denB = oB[0:64]; swap halves on the PE so
            # each reciprocal lands on its numerator's partitions.  High
            # priority so the scr copies jump the DVE queue (the PE's den
            # matmul and the next pair's accumulator reuse both wait on
            # this path).
            with tc.high_priority(offset=165):
                scr = nrm.tile([128, 512], BF16, tag="scr", name=f"scr{p}{j}")
                nc.vector.tensor_copy(scr[64:128, :], oA[64:128, :])
                nc.vector.tensor_copy(scr[0:64, :], oB[0:64, :])
                den2 = rot.tile([128, 512], F32, tag="r", name=f"den{p}{j}")
                nc.tensor.matmul(den2, swap_sb, scr, start=True, stop=True)
                rec = nrm.tile([128, 512], F32, tag="rec", name=f"rec{p}{j}")
                nc.vector.reciprocal_approx_fast(out=rec, in_=den2)
            for h in range(halves):
                hw = 512 // halves
                csl = slice(j * 512 + h * hw, j * 512 + (h + 1) * hw)
                osl = slice(h * hw, (h + 1) * hw)
                nc.vector.tensor_mul(attnT[p][0:64, csl], oA[0:64, osl],
                                     rec[0:64, osl])
                nc.vector.tensor_mul(attnT[p][64:128, csl], oB[64:128, osl],
                                     rec[64:128, osl])
                if after_half is not None:
                    after_half(h)

        # ---- main schedule ----
        qk_interleaved(0, 0)
        for u in proj_units(0, kinds="qk", pairs=[1]) + proj_units(0, kinds="v"):
            u()
        # Filler assignment keeps every step PE-bound.  Block 3's K and V
        # projections are only consumed from chunk 12 of step 3, so they
        # slide into step 3 itself as guaranteed-ready PE filler for its
        # exp-heavy stretch; out-projections lag two steps for the same
        # reason.
        for j in range(NJ):
            if j == 0:
                fillers = proj_units(1)
            elif j == 1:
                fillers = proj_units(2) + psf_units(0)
            elif j == 2:
                fillers = proj_units(3, kinds="q") + psf_units(1)
            else:
                # K3/V3 lead: they are consumed by this step's own chunks
                # from c=12.
                fillers = proj_units(3, kinds="kv") + psf_units(2)
            nch_j = 4 * j + 4
            total_chunks = 2 * nch_j
            # Finish fillers a few chunks early so their PSUM drains don't
            # jam the DVE right when the tail norm path needs it.
            denom = total_chunks if j == 0 else max(1, total_chunks - 4)
            done = 0
            ci = 0
            for p in range(NPAIR):
                oA = op.tile([128, 512], F32, tag="oA", name=f"oA{p}{j}")
                oB = op.tile([128, 512], F32, tag="oB", name=f"oB{p}{j}")
                pend = []
                for c in range(nch_j):
                    pend.append((c,) + emit_scores(p, j, c))
                    if len(pend) > 3:
                        c0, p2, fo, w = pend.pop(0)
                        emit_attnv(p, j, c0, oA, oB, nch_j - 1, p2, fo, w)
                    ci += 1
                    want = min(len(fillers), ci * len(fillers) // denom)
                    while done < want:
                        fillers[done]()
                        done += 1
                for c0, p2, fo, w in pend:
                    emit_attnv(p, j, c0, oA, oB, nch_j - 1, p2, fo, w)
                if j == NJ - 1 and p == NPAIR - 1:
                    # keep the PE (and its HAM clock) busy through the
                    # norm chain's DVE round-trip so the tail out-proj
                    # matmuls run at 2.4 GHz
                    for i in range(4):
                        pw = rot.tile([128, 512], F32, tag="r",
                                      name=f"tailwup{i}")
                        nc.tensor.matmul(pw, wup[:, 0:128], wup,
                                         start=True, stop=True)

                    def tail_half(h, j=j):
                        for u in psf_units(j, tail=True, qs=(4 * j + h,)):
                            u()
                    emit_norm(p, j, oA, oB, halves=4, after_half=tail_half)
                else:
                    emit_norm(p, j, oA, oB)
            while done < len(fillers):
                fillers[done]()
                done += 1

        if _DEBUG:
            for p in range(NPAIR):
                nc.sync.dma_start(out=d[f"dbg_qt{p}"][:], in_=qt_sb[p].bitcast(F32))
                nc.sync.dma_start(out=d[f"dbg_kt{p}"][:], in_=kt_sb[p].bitcast(F32))
                nc.sync.dma_start(out=d[f"dbg_at{p}"][:], in_=attnT[p].bitcast(F32))


_NC_CACHE = {}
_DEBUG = False


def _get_nc():
    key = "nc"
    if key not in _NC_CACHE:
        nc = bacc.Bacc(None, target_bir_lowering=False)
        d = {
            "xt": nc.dram_tensor("xt", [128, NJ * NCH * 512], BF16,
                                 kind="ExternalInput"),
            "wqq": nc.dram_tensor("wqq", [NPAIR, 128, NCH * 128], BF16,
                                  kind="ExternalInput"),
            "wkk": nc.dram_tensor("wkk", [NPAIR, 128, NCH * 128], BF16,
                                  kind="ExternalInput"),
            "wv": nc.dram_tensor("wv", [128, NCH * 256], BF16, kind="ExternalInput"),
            "wo": nc.dram_tensor("wo", [128, 2 * 1024], BF16, kind="ExternalInput"),
            "smalls": nc.dram_tensor("smalls", [128, _SMALLW], F32,
                                     kind="ExternalInput"),
            "swapmd": nc.dram_tensor("swapmd", [128, _SWMDW], BF16,
                                     kind="ExternalInput"),
            "y": nc.dram_tensor("y", [S, H], BF16, kind="ExternalOutput"),
        }
        if _DEBUG:
            for p in range(NPAIR):
                d[f"dbg_qt{p}"] = nc.dram_tensor(f"dbg_qt{p}", [128, S], F32,
                                                 kind="ExternalOutput")
                d[f"dbg_kt{p}"] = nc.dram_tensor(f"dbg_kt{p}", [128, S], F32,
                                                 kind="ExternalOutput")
                d[f"dbg_at{p}"] = nc.dram_tensor(f"dbg_at{p}", [128, S], F32,
                                                 kind="ExternalOutput")
        _emit(nc, d)
        nc.finalize()
        _NC_CACHE[key] = nc
    return _NC_CACHE[key]


def _chunked(w, ncols):
    """[H, ncols] -> [128, NCH*ncols] with chunk c of rows at cols c*ncols."""
    return np.ascontiguousarray(
        w.reshape(NCH, 128, ncols).transpose(1, 0, 2).reshape(128, NCH * ncols))


def _make_in_maps(batch, input_ids, W_Q, W_K, W_V, W_O, b_Q, b_K):
    m = np.triu(np.ones((128, 128), np.float32))
    mdiag2 = np.stack([m, m], axis=1).reshape(128, 256)  # [128, 2*128]
    swap = np.zeros((128, 128), np.float32)
    swap[64:128, 0:64] = np.eye(64, dtype=np.float32)
    swap[0:64, 64:128] = np.eye(64, dtype=np.float32)
    bf = ml_dtypes.bfloat16
    in_maps = []
    for core in range(NCORE):
        b, g = divmod(core, 4)
        base = 256 * g  # first feature column of this core's 4 heads
        wqq = np.stack([_chunked(W_Q[:, base + 128 * p: base + 128 * (p + 1)], 128)
                        for p in range(NPAIR)])
        wkk = np.stack([_chunked(W_K[:, base + 128 * p: base + 128 * (p + 1)], 128)
                        for p in range(NPAIR)])
        wv = _chunked(W_V[:, base: base + 256], 256)
        wo = np.ascontiguousarray(
            W_O[base: base + 256, :].reshape(2, 128, H)
            .transpose(1, 0, 2).reshape(128, 2 * H))
        bq = np.stack([b_Q[base + 128 * p: base + 128 * (p + 1)]
                       for p in range(NPAIR)], axis=1)
        bk = np.stack([b_K[base + 128 * p: base + 128 * (p + 1)]
                       for p in range(NPAIR)], axis=1)
        keep = input_ids[b] != 0
        nbias = np.where(keep, 0.0, NEG_BIAS).astype(np.float32)
        nbias = nbias.reshape(NKC, 128).T
        smalls = np.concatenate([bq, bk, nbias], axis=1)
        assert smalls.shape == (128, _SMALLW)
        swapmd = np.concatenate([swap, mdiag2], axis=1).astype(bf)
        assert swapmd.shape == (128, _SWMDW)
        # x^T block-major: [128, block j, chunk c, 512]
        xt = batch[b].T.reshape(NCH, 128, NJ, 512).transpose(1, 2, 0, 3)
        xt = np.ascontiguousarray(xt).reshape(128, NJ * NCH * 512)
        in_maps.append({
            "xt": xt.astype(bf), "wqq": wqq.astype(bf),
            "wkk": wkk.astype(bf), "wv": wv.astype(bf),
            "wo": wo.astype(bf),
            "smalls": np.ascontiguousarray(smalls.astype(np.float32)),
            "swapmd": np.ascontiguousarray(swapmd),
        })
    return in_maps


def _run(in_maps, **kwargs):
    nc = _get_nc()
    return run_bass_kernel_spmd(nc, in_maps, core_ids=list(range(NCORE)), **kwargs)


def kernel(batch, input_ids, W_Q, W_K, W_V, b_Q, b_K, b_V, W_O, b_O,
           _results_out=None, **run_kwargs):
    batch = np.asarray(batch, np.float32)
    input_ids = np.asarray(input_ids)
    W_Q, W_K, W_V = (np.asarray(a, np.float32) for a in (W_Q, W_K, W_V))
    b_Q, b_K, b_V = (np.asarray(a, np.float32) for a in (b_Q, b_K, b_V))
    W_O = np.asarray(W_O, np.float32)
    b_O = np.asarray(b_O, np.float32)

    in_maps = _make_in_maps(batch, input_ids, W_Q, W_K, W_V, W_O, b_Q, b_K)
    res = _run(in_maps, **run_kwargs)
    if _results_out is not None:
        _results_out.append(res)
    ys = [np.asarray(res.results[c]["y"], np.float32) for c in range(NCORE)]
    out = np.stack([sum(ys[4 * b: 4 * b + 4]) for b in range(B)], axis=0)
    # b_V enters as attn@1 * b_V = b_V (softmax rows sum to 1), then @ W_O.
    const_row = (b_V @ W_O + b_O).astype(np.float32)
    return (out + const_row).astype(np.float32)
